# revision 6
# baseline (speedup 1.0000x reference)
"""Trainium2 Bass kernel for nn_C3k_CBSA (landmark/CBSA sparse attention block).

Strategy: data-parallel over batch B=8 across 8 NeuronCores (one batch element
per core, zero collectives). Per core the whole block is fused into one Bass
kernel: cv1/cv2 1x1 convs + SiLU, landmark pooling, landmark<->token cross
attention, landmark self attention, scatter-back, output projection, cv3.

Key algebraic restructurings (all exact up to fp assoc.):
  - logits = rep_h.T @ proj_h = (proj_w @ rep_cm).T @ y1  -> proj never
    materialized over tokens; only a tiny per-pair Q = pw.T @ rep_cm.
  - rep = pool(proj) = proj_w @ pool(y1): pooling commutes with 1x1 conv.
  - rep_delta = (E @ y1.T) @ proj_w.T with E transposed chunkwise on PE.
  - softmax 1/Z and step_x folded into landmark-sized tensors (E stays
    unnormalized); scatter-back is G'.T @ E with stacked-landmark contraction.

Head pairing packs two 64-dim heads into 128 partitions with block-diagonal
stationary operands so every matmul uses the full PE array. Emission is
software-pipelined (lag-one chunk) so each engine's in-order queue never
stalls on the previous chunk's cross-engine dependency.
"""

import os
import numpy as np
import ml_dtypes

try:
    import concourse  # noqa: F401
except ImportError:  # fresh grading dir: fall back to the staged repo path
    import sys

    for p in ("/opt/trn_rl_repo", "/root/.axon_site/_ro/trn_rl_repo"):
        if os.path.isdir(p):
            sys.path.insert(0, p)
            break

import concourse.bass as bass
import concourse.mybir as mybir
import concourse.tile as tile
from concourse import bacc
from concourse.bass import ts
from concourse.bass_utils import run_bass_kernel_spmd
from concourse.masks import make_identity

F32 = mybir.dt.float32
BF16 = mybir.dt.bfloat16
AF = mybir.ActivationFunctionType
ALU = mybir.AluOpType

B, C1, C2, H, W = 8, 256, 256, 80, 80
C_ = 128
HEADS, DH = 8, 64
INNER = HEADS * DH  # 512
SCALE = DH ** -0.5
N = H * W  # 6400
NPAIRS = HEADS // 2  # 4 head-pair groups of 128 partitions

CHUNKS = [(i * 512, min(512, N - i * 512)) for i in range((N + 511) // 512)]
NC_ = len(CHUNKS)  # 13
NT = N // 128  # 50 token chunks of 128


def _build(step_rep: np.ndarray, step_x: np.ndarray) -> bass.Bass:
    nc = bacc.Bacc("TRN2", target_bir_lowering=False, debug=False, num_devices=8)

    x_d = nc.dram_tensor("x", [C1, N], BF16, kind="ExternalInput")
    w1_d = nc.dram_tensor("w1t", [C1, C_], BF16, kind="ExternalInput")
    b1_d = nc.dram_tensor("b1", [C_, 1], F32, kind="ExternalInput")
    w2_d = nc.dram_tensor("w2t", [C1, C_], BF16, kind="ExternalInput")
    b2_d = nc.dram_tensor("b2", [C_, 1], F32, kind="ExternalInput")
    w3_d = nc.dram_tensor("w3t", [2 * C_, C2], BF16, kind="ExternalInput")
    b3_d = nc.dram_tensor("b3", [C2, 1], F32, kind="ExternalInput")
    pw_d = nc.dram_tensor("pwt", [C_, INNER], BF16, kind="ExternalInput")
    pwo_d = nc.dram_tensor("pwo", [INNER, C_], BF16, kind="ExternalInput")
    ow_d = nc.dram_tensor("owt", [INNER, C_], BF16, kind="ExternalInput")
    ob_d = nc.dram_tensor("outb", [C_, 1], F32, kind="ExternalInput")
    out_d = nc.dram_tensor("out", [C2, N], F32, kind="ExternalOutput")

    sr = [float(v) for v in np.asarray(step_rep).reshape(-1)]
    sx = [float(v) for v in np.asarray(step_x).reshape(-1)]

    def subchunks(ci):
        c0, w = CHUNKS[ci]
        return range(c0 // 128, (c0 + w) // 128)

    with tile.TileContext(nc) as tc:
        with (
            tc.tile_pool(name="const", bufs=1) as cp,
            tc.tile_pool(name="persist", bufs=1) as pp,
            tc.tile_pool(name="xin", bufs=NC_) as xp,
            tc.tile_pool(name="etm", bufs=3) as ep,
            tc.tile_pool(name="outs", bufs=3) as op_,
            tc.tile_pool(name="pmain", bufs=3, space="PSUM") as pm,
            tc.tile_pool(name="ptp", bufs=2, space="PSUM") as ptp,
            tc.tile_pool(name="pacc", bufs=1, space="PSUM") as pacc,
            tc.tile_pool(name="psmall", bufs=2, space="PSUM") as ps,
        ):
            # ---- constants ----
            w1_t = cp.tile([128, 2, C_], BF16, tag="w1")
            w2_t = cp.tile([128, 2, C_], BF16, tag="w2")
            w3_t = cp.tile([128, 2, C2], BF16, tag="w3")
            pw_t = cp.tile([128, INNER], BF16, tag="pw")
            pwo_t = cp.tile([128, 4, C_], BF16, tag="pwo")
            ow_t = cp.tile([128, 4, C_], BF16, tag="ow")
            b1_t = cp.tile([128, 1], F32, tag="b1")
            b2_t = cp.tile([128, 1], F32, tag="b2")
            b3_t = cp.tile([128, 2, 1], F32, tag="b3")
            ob_t = cp.tile([128, 1], F32, tag="ob")
            id_bf = cp.tile([128, 128], BF16, tag="idb")
            id_f32 = cp.tile([128, 128], F32, tag="idf")

            for j in range(2):
                nc.sync.dma_start(w1_t[:, j, :], w1_d[ts(j, 128), :])
                nc.sync.dma_start(w2_t[:, j, :], w2_d[ts(j, 128), :])
                nc.sync.dma_start(w3_t[:, j, :], w3_d[ts(j, 128), :])
                nc.sync.dma_start(b3_t[:, j, :], b3_d[ts(j, 128), :])
            for j in range(4):
                nc.sync.dma_start(ow_t[:, j, :], ow_d[ts(j, 128), :])
                nc.sync.dma_start(pwo_t[:, j, :], pwo_d[ts(j, 128), :])
            nc.sync.dma_start(pw_t[:], pw_d[:, :])
            nc.sync.dma_start(b1_t[:], b1_d[:, :])
            nc.sync.dma_start(b2_t[:], b2_d[:, :])
            nc.sync.dma_start(ob_t[:], ob_d[:, :])
            make_identity(nc, id_bf[:])
            make_identity(nc, id_f32[:])

            # ---- persistent activations ----
            y1_t = pp.tile([128, N], BF16, tag="y1")
            y2_t = pp.tile([128, N], BF16, tag="y2")
            y1tm_t = pp.tile([128, N], BF16, tag="y1tm")
            e_t = pp.tile([128, NPAIRS, N], BF16, tag="elm")
            ycb_t = pp.tile([128, N], BF16, tag="ycb")
            zpart_t = pp.tile([128, NPAIRS, NC_], F32, tag="zpart")
            rinv_t = pp.tile([128, NPAIRS], F32, tag="rinv")

            # ---- phase A (pipelined): cv1 + token-major transpose of y1 ----
            def cv1_chunk(ci):
                c0, w = CHUNKS[ci]
                x_c = xp.tile([128, 2, 512], BF16, tag="x")
                for j in range(2):
                    nc.sync.dma_start(x_c[:, j, :w], x_d[ts(j, 128), c0 : c0 + w])
                p1 = pm.tile([128, 512], F32, tag="pm")
                nc.tensor.matmul(p1[:, :w], w1_t[:, 0, :], x_c[:, 0, :w], start=True, stop=False)
                nc.tensor.matmul(p1[:, :w], w1_t[:, 1, :], x_c[:, 1, :w], start=False, stop=True)
                nc.scalar.activation(y1_t[:, c0 : c0 + w], p1[:, :w], AF.Silu, bias=b1_t[:])
                return x_c

            def y1tm_chunk(ci):
                tp = ptp.tile([128, 512], BF16, tag="tp")
                sub = list(subchunks(ci))
                for k, t in enumerate(sub):
                    nc.tensor.transpose(tp[:, ts(k, 128)], y1_t[:, ts(t, 128)], id_bf[:])
                nc.vector.tensor_copy(
                    y1tm_t[:, sub[0] * 128 : (sub[-1] + 1) * 128], tp[:, : len(sub) * 128]
                )

            xs = {}
            for ci in range(NC_):
                xs[ci] = cv1_chunk(ci)
                if ci > 0:
                    y1tm_chunk(ci - 1)
            y1tm_chunk(NC_ - 1)

            # ---- pooling -> rep -> rep_cm -> Q ----
            pool1 = pp.tile([128, 640], F32, tag="pool1")
            nc.vector.tensor_reduce(
                pool1[:],
                y1_t[:].rearrange("p (rw kw c) -> p rw kw c", rw=80, kw=8, c=10),
                axis=mybir.AxisListType.X,
                op=ALU.add,
            )
            pool2 = pp.tile([128, 64], F32, tag="pool2")
            nc.vector.tensor_reduce(
                pool2[:],
                pool1[:].rearrange("p (kh r kw) -> p kh kw r", kh=8, r=10, kw=8),
                axis=mybir.AxisListType.X,
                op=ALU.add,
            )
            y1pool_bf = pp.tile([128, 64], BF16, tag="y1pool")
            nc.vector.tensor_scalar_mul(y1pool_bf[:], pool2[:], 1.0 / 100.0)

            rep_ps = pm.tile([64, 512], F32, tag="pm")
            nc.tensor.matmul(rep_ps[:], y1pool_bf[:], pw_t[:], start=True, stop=True)
            rep_f32 = pp.tile([64, 512], F32, tag="repf")
            rep_bf = pp.tile([64, 512], BF16, tag="repb")
            nc.vector.tensor_copy(rep_f32[:], rep_ps[:])
            nc.vector.tensor_copy(rep_bf[:], rep_ps[:])

            repcm_bd, repcm_f, q_bf = [], [], []
            for pr in range(NPAIRS):
                tpb = ps.tile([128, 128], BF16, tag="lm")
                nc.tensor.transpose(tpb[:, :64], rep_bf[:, ts(pr, 128)], id_bf[:64, :64])
                bd = pp.tile([128, 128], BF16, tag=f"repbd{pr}")
                nc.gpsimd.memset(bd[:], 0.0)
                nc.vector.tensor_copy(bd[0:64, 0:64], tpb[0:64, :64])
                nc.vector.tensor_copy(bd[64:128, 64:128], tpb[64:128, :64])
                repcm_bd.append(bd)

                tpf = ps.tile([128, 128], F32, tag="lm")
                nc.tensor.transpose(tpf[:, :64], rep_f32[:, ts(pr, 128)], id_f32[:64, :64])
                rcf = pp.tile([128, 64], F32, tag=f"repcf{pr}")
                nc.vector.tensor_copy(rcf[:], tpf[:, :64])
                repcm_f.append(rcf)

            for pr in range(NPAIRS):
                qp = ps.tile([128, 128], F32, tag="lm")
                nc.tensor.matmul(qp[:], pwo_t[:, pr, :], repcm_bd[pr][:], start=True, stop=True)
                q = pp.tile([128, 128], BF16, tag=f"q{pr}")
                nc.vector.tensor_copy(q[:], qp[:])
                q_bf.append(q)

            # ---- phase B (pipelined): logits+exp, cv2, E-transpose + T accum ----
            t_acc = pacc.tile([128, NPAIRS, 128], F32, tag="tacc")

            def logits_chunk(ci):
                c0, w = CHUNKS[ci]
                for pr in range(NPAIRS):
                    pl = pm.tile([128, 512], F32, tag="pm")
                    nc.tensor.matmul(pl[:, :w], q_bf[pr][:], y1_t[:, c0 : c0 + w], start=True, stop=True)
                    nc.scalar.activation(
                        e_t[:, pr, c0 : c0 + w],
                        pl[:, :w],
                        AF.Exp,
                        scale=SCALE,
                        accum_out=zpart_t[:, pr, ci : ci + 1],
                    )

            def cv2_chunk(ci):
                c0, w = CHUNKS[ci]
                x_c = xs.pop(ci)
                p2 = pm.tile([128, 512], F32, tag="pm")
                nc.tensor.matmul(p2[:, :w], w2_t[:, 0, :], x_c[:, 0, :w], start=True, stop=False)
                nc.tensor.matmul(p2[:, :w], w2_t[:, 1, :], x_c[:, 1, :w], start=False, stop=True)
                nc.scalar.activation(y2_t[:, c0 : c0 + w], p2[:, :w], AF.Silu, bias=b2_t[:])

            def etp_chunk(ci):
                for t in subchunks(ci):
                    tpe = ptp.tile([128, 512], BF16, tag="tp")
                    for pr in range(NPAIRS):
                        nc.tensor.transpose(tpe[:, ts(pr, 128)], e_t[:, pr, ts(t, 128)], id_bf[:])
                    etm = ep.tile([128, 512], BF16, tag="etm")
                    nc.vector.tensor_copy(etm[:], tpe[:])
                    for pr in range(NPAIRS):
                        nc.tensor.matmul(
                            t_acc[:, pr, :],
                            etm[:, ts(pr, 128)],
                            y1tm_t[:, ts(t, 128)],
                            start=(t == 0),
                            stop=(t == NT - 1),
                        )

            for ci in range(NC_):
                logits_chunk(ci)
                cv2_chunk(ci)
                if ci > 0:
                    etp_chunk(ci - 1)
            etp_chunk(NC_ - 1)

            # ---- softmax denominators ----
            for pr in range(NPAIRS):
                nc.vector.tensor_reduce(
                    rinv_t[:, pr : pr + 1], zpart_t[:, pr, :], axis=mybir.AxisListType.X, op=ALU.add
                )
            nc.vector.reciprocal(rinv_t[:], rinv_t[:])

            # ---- landmark-sized attention core (4 pairs interleaved stepwise) ----
            def lm_sb(tag, dtype=BF16):
                return [pp.tile([128, 128], dtype, tag=f"{tag}{pr}", name=f"{tag}{pr}") for pr in range(NPAIRS)]

            tn, tnt, rep2, rep2b = lm_sb("tn"), lm_sb("tnt"), lm_sb("rep2", F32), lm_sb("rep2b")
            e2, e2n, e2t = lm_sb("e2", F32), lm_sb("e2n"), lm_sb("e2t")
            r2l, xd, g_bf = lm_sb("r2l"), lm_sb("xd"), lm_sb("g")
            z2 = [pp.tile([128, 1], F32, tag=f"z2{pr}", name=f"z2{pr}") for pr in range(NPAIRS)]

            for pr in range(NPAIRS):  # rinv * T  (k-double rows)
                nc.vector.tensor_scalar(
                    tn[pr][:], t_acc[:, pr, :], rinv_t[:, pr : pr + 1], None, op0=ALU.mult
                )
            tnt_ps = [ps.tile([128, 128], BF16, tag="lm", name=f"tnt_ps{pr}") for pr in range(NPAIRS)]
            for pr in range(NPAIRS):
                nc.tensor.transpose(tnt_ps[pr][:], tn[pr][:], id_bf[:])
            for pr in range(NPAIRS):
                nc.vector.tensor_copy(tnt[pr][:], tnt_ps[pr][:])
            rd_ps = [ps.tile([128, 128], F32, tag="lm", name=f"rd_ps{pr}") for pr in range(NPAIRS)]
            for pr in range(NPAIRS):  # rep_delta channel-major
                nc.tensor.matmul(rd_ps[pr][:], pw_t[:, ts(pr, 128)], tnt[pr][:], start=True, stop=True)
            for pr in range(NPAIRS):
                nc.gpsimd.memset(rep2[pr][:], 0.0)
            for pr in range(NPAIRS):
                for q in range(2):
                    qs = slice(64 * q, 64 * (q + 1))
                    nc.vector.scalar_tensor_tensor(
                        rep2[pr][qs, qs], rd_ps[pr][qs, qs], sr[2 * pr + q],
                        repcm_f[pr][qs, :], op0=ALU.mult, op1=ALU.add,
                    )
                nc.vector.tensor_copy(rep2b[pr][:], rep2[pr][:])
            l2_ps = [ps.tile([128, 128], F32, tag="lm", name=f"l2_ps{pr}") for pr in range(NPAIRS)]
            for pr in range(NPAIRS):
                nc.tensor.matmul(l2_ps[pr][:], rep2b[pr][:], rep2b[pr][:], start=True, stop=True)
            for pr in range(NPAIRS):
                nc.scalar.activation(e2[pr][:], l2_ps[pr][:], AF.Exp, scale=SCALE)
            for pr in range(NPAIRS):
                nc.vector.tensor_reduce(z2[pr][0:64, :], e2[pr][0:64, 0:64], axis=mybir.AxisListType.X, op=ALU.add)
                nc.vector.tensor_reduce(z2[pr][64:128, :], e2[pr][64:128, 64:128], axis=mybir.AxisListType.X, op=ALU.add)
                nc.vector.reciprocal(z2[pr][:], z2[pr][:])
            for pr in range(NPAIRS):
                for q in range(2):
                    qs = slice(64 * q, 64 * (q + 1))
                    nc.vector.tensor_scalar(
                        e2n[pr][qs, :], e2[pr][qs, :], z2[pr][qs, :], sx[2 * pr + q],
                        op0=ALU.mult, op1=ALU.mult,
                    )
            e2t_ps = [ps.tile([128, 128], BF16, tag="lm", name=f"e2t_ps{pr}") for pr in range(NPAIRS)]
            r2l_ps = [ps.tile([128, 128], BF16, tag="lm", name=f"r2l_ps{pr}") for pr in range(NPAIRS)]
            for pr in range(NPAIRS):
                nc.tensor.transpose(e2t_ps[pr][:], e2n[pr][:], id_bf[:])
                nc.tensor.transpose(r2l_ps[pr][:], rep2b[pr][:], id_bf[:])
            for pr in range(NPAIRS):
                nc.gpsimd.memset(e2t[pr][:], 0.0)
                for q in range(2):
                    qs = slice(64 * q, 64 * (q + 1))
                    nc.vector.tensor_copy(e2t[pr][qs, qs], e2t_ps[pr][qs, qs])
                nc.vector.tensor_copy(r2l[pr][:], r2l_ps[pr][:])
            xd_ps = [ps.tile([128, 128], F32, tag="lm", name=f"xd_ps{pr}") for pr in range(NPAIRS)]
            for pr in range(NPAIRS):  # x_delta channel-major (block-diag)
                nc.tensor.matmul(xd_ps[pr][:], r2l[pr][:], e2t[pr][:], start=True, stop=True)
            for pr in range(NPAIRS):
                nc.vector.tensor_copy(xd[pr][:], xd_ps[pr][:])
            g_ps = [ps.tile([128, 128], F32, tag="lm", name=f"g_ps{pr}") for pr in range(NPAIRS)]
            for pr in range(NPAIRS):
                nc.tensor.matmul(g_ps[pr][:], xd[pr][:], ow_t[:, pr, :], start=True, stop=True)
            for pr in range(NPAIRS):
                nc.vector.tensor_scalar(
                    g_bf[pr][:], g_ps[pr][:], rinv_t[:, pr : pr + 1], None, op0=ALU.mult
                )

            # ---- phase C (pipelined): scatter + bias, then cv3 + SiLU + out ----
            def scatter_chunk(ci):
                c0, w = CHUNKS[ci]
                sc = pm.tile([128, 512], F32, tag="pm")
                for pr in range(NPAIRS):
                    nc.tensor.matmul(
                        sc[:, :w], g_bf[pr][:], e_t[:, pr, c0 : c0 + w],
                        start=(pr == 0), stop=(pr == NPAIRS - 1),
                    )
                nc.vector.tensor_scalar(
                    ycb_t[:, c0 : c0 + w], sc[:, :w], ob_t[:], None, op0=ALU.add
                )

            def cv3_chunk(ci):
                c0, w = CHUNKS[ci]
                for co in range(2):
                    po = pm.tile([128, 512], F32, tag="pm")
                    nc.tensor.matmul(po[:, :w], w3_t[:, 0, ts(co, 128)], ycb_t[:, c0 : c0 + w], start=True, stop=False)
                    nc.tensor.matmul(po[:, :w], w3_t[:, 1, ts(co, 128)], y2_t[:, c0 : c0 + w], start=False, stop=True)
                    ot = op_.tile([128, 512], F32, tag="ot")
                    nc.scalar.activation(ot[:, :w], po[:, :w], AF.Silu, bias=b3_t[:, co, :])
                    nc.sync.dma_start(out_d[ts(co, 128), c0 : c0 + w], ot[:, :w])

            for ci in range(NC_):
                scatter_chunk(ci)
                if ci > 0:
                    cv3_chunk(ci - 1)
            cv3_chunk(NC_ - 1)

    nc.finalize()
    return nc


_CACHE: dict = {}


def _get_nc(step_rep, step_x):
    key = (tuple(np.asarray(step_rep).reshape(-1).tolist()),
           tuple(np.asarray(step_x).reshape(-1).tolist()))
    if key not in _CACHE:
        _CACHE[key] = _build(step_rep, step_x)
    return _CACHE[key]


def run(inputs: dict, trace: bool = False, tmpdir: str | None = None):
    bf = ml_dtypes.bfloat16
    x = np.asarray(inputs["x"], np.float32).reshape(B, C1, N)

    def prep(a):
        return np.ascontiguousarray(np.asarray(a, np.float32)).astype(bf)

    w1t = prep((np.asarray(inputs["cv1_s"], np.float32)[:, None] * np.asarray(inputs["cv1_w"], np.float32)).T)
    w2t = prep((np.asarray(inputs["cv2_s"], np.float32)[:, None] * np.asarray(inputs["cv2_w"], np.float32)).T)
    w3t = prep((np.asarray(inputs["cv3_s"], np.float32)[:, None] * np.asarray(inputs["cv3_w"], np.float32)).T)
    pwt = prep(np.asarray(inputs["proj_w"], np.float32).T)
    pwo = prep(np.asarray(inputs["proj_w"], np.float32))
    owt = prep(np.asarray(inputs["out_w"], np.float32).T)
    b1 = np.ascontiguousarray(np.asarray(inputs["cv1_b"], np.float32).reshape(C_, 1))
    b2 = np.ascontiguousarray(np.asarray(inputs["cv2_b"], np.float32).reshape(C_, 1))
    b3 = np.ascontiguousarray(np.asarray(inputs["cv3_b"], np.float32).reshape(C2, 1))
    ob = np.ascontiguousarray(np.asarray(inputs["out_b"], np.float32).reshape(C_, 1))

    nc = _get_nc(inputs["step_rep"], inputs["step_x"])

    in_maps = []
    for b in range(B):
        in_maps.append(
            {
                "x": np.ascontiguousarray(x[b].astype(bf)),
                "w1t": w1t, "b1": b1,
                "w2t": w2t, "b2": b2,
                "w3t": w3t, "b3": b3,
                "pwt": pwt, "pwo": pwo, "owt": owt, "outb": ob,
            }
        )

    res = run_bass_kernel_spmd(
        nc, in_maps, core_ids=list(range(B)), trace=trace, tmpdir=tmpdir
    )
    out = np.stack([np.asarray(res.results[b]["out"], np.float32) for b in range(B)])
    return out.reshape(B, C2, H, W), res


def kernel(**inputs) -> np.ndarray:
    out, _ = run(inputs, trace=False)
    return out


# revision 8
# speedup vs baseline: 1.1754x; 1.1754x over previous
"""Trainium2 Bass kernel for nn_C3k_CBSA (landmark/CBSA sparse attention block).

Strategy: data-parallel over batch B=8 across 8 NeuronCores (one batch element
per core, zero collectives). Per core the whole block is fused into one Bass
kernel: cv1/cv2 1x1 convs + SiLU, landmark pooling, landmark<->token cross
attention, landmark self attention, scatter-back, output projection, cv3.

Key algebraic restructurings (all exact up to fp assoc.):
  - logits = rep_h.T @ proj_h = (proj_w @ rep_cm).T @ y1  -> proj never
    materialized over tokens; only a tiny per-pair Q = pw.T @ rep_cm.
  - rep = pool(proj) = proj_w @ pool(y1): pooling commutes with 1x1 conv.
  - rep_delta = (E @ y1.T) @ proj_w.T with E transposed chunkwise on PE.
  - softmax 1/Z and step_x folded into landmark-sized tensors (E stays
    unnormalized); scatter-back is G'.T @ E with stacked-landmark contraction.

Head pairing packs two 64-dim heads into 128 partitions with block-diagonal
stationary operands so every matmul uses the full PE array. Emission is
software-pipelined (lag-one chunk) so each engine's in-order queue never
stalls on the previous chunk's cross-engine dependency.
"""

import os
import numpy as np
import ml_dtypes

try:
    import concourse  # noqa: F401
except ImportError:  # fresh grading dir: fall back to the staged repo path
    import sys

    for p in ("/opt/trn_rl_repo", "/root/.axon_site/_ro/trn_rl_repo"):
        if os.path.isdir(p):
            sys.path.insert(0, p)
            break

import concourse.bass as bass
import concourse.mybir as mybir
import concourse.tile as tile
from concourse import bacc
from concourse.bass import ts
from concourse.bass_utils import run_bass_kernel_spmd
from concourse.masks import make_identity

F32 = mybir.dt.float32
BF16 = mybir.dt.bfloat16
AF = mybir.ActivationFunctionType
ALU = mybir.AluOpType

B, C1, C2, H, W = 8, 256, 256, 80, 80
C_ = 128
HEADS, DH = 8, 64
INNER = HEADS * DH  # 512
SCALE = DH ** -0.5
N = H * W  # 6400
NPAIRS = HEADS // 2  # 4 head-pair groups of 128 partitions

CHUNKS = [(i * 512, min(512, N - i * 512)) for i in range((N + 511) // 512)]
NC_ = len(CHUNKS)  # 13
NT = N // 128  # 50 token chunks of 128


def _build(step_rep: np.ndarray, step_x: np.ndarray) -> bass.Bass:
    nc = bacc.Bacc("TRN2", target_bir_lowering=False, debug=False, num_devices=8)

    x_d = nc.dram_tensor("x", [C1, N], BF16, kind="ExternalInput")
    w1_d = nc.dram_tensor("w1t", [C1, C_], BF16, kind="ExternalInput")
    b1_d = nc.dram_tensor("b1", [C_, 1], F32, kind="ExternalInput")
    w2_d = nc.dram_tensor("w2t", [C1, C_], BF16, kind="ExternalInput")
    b2_d = nc.dram_tensor("b2", [C_, 1], F32, kind="ExternalInput")
    w3_d = nc.dram_tensor("w3t", [2 * C_, C2], BF16, kind="ExternalInput")
    b3_d = nc.dram_tensor("b3", [C2, 1], F32, kind="ExternalInput")
    pw_d = nc.dram_tensor("pwt", [C_, INNER], BF16, kind="ExternalInput")
    pwo_d = nc.dram_tensor("pwo", [INNER, C_], BF16, kind="ExternalInput")
    ow_d = nc.dram_tensor("owt", [INNER, C_], BF16, kind="ExternalInput")
    ob_d = nc.dram_tensor("outb", [C_, 1], F32, kind="ExternalInput")
    out_d = nc.dram_tensor("out", [C2, N], F32, kind="ExternalOutput")

    sr = [float(v) for v in np.asarray(step_rep).reshape(-1)]
    sx = [float(v) for v in np.asarray(step_x).reshape(-1)]

    def subchunks(ci):
        c0, w = CHUNKS[ci]
        return range(c0 // 128, (c0 + w) // 128)

    with tile.TileContext(nc) as tc:
        with (
            tc.tile_pool(name="const", bufs=1) as cp,
            tc.tile_pool(name="persist", bufs=1) as pp,
            tc.tile_pool(name="xin", bufs=3) as xp,
            tc.tile_pool(name="etm", bufs=3) as ep,
            tc.tile_pool(name="outs", bufs=3) as op_,
            tc.tile_pool(name="pmain", bufs=3, space="PSUM") as pm,
            tc.tile_pool(name="ptp", bufs=2, space="PSUM") as ptp,
            tc.tile_pool(name="pacc", bufs=1, space="PSUM") as pacc,
            tc.tile_pool(name="psmall", bufs=2, space="PSUM") as ps,
        ):
            # ---- constants ----
            w1_t = cp.tile([128, 2, C_], BF16, tag="w1")
            w2_t = cp.tile([128, 2, C_], BF16, tag="w2")
            w3_t = cp.tile([128, 2, C2], BF16, tag="w3")
            pw_t = cp.tile([128, INNER], BF16, tag="pw")
            pwo_t = cp.tile([128, 4, C_], BF16, tag="pwo")
            ow_t = cp.tile([128, 4, C_], BF16, tag="ow")
            b1_t = cp.tile([128, 1], F32, tag="b1")
            b2_t = cp.tile([128, 1], F32, tag="b2")
            b3_t = cp.tile([128, 2, 1], F32, tag="b3")
            ob_t = cp.tile([128, 1], F32, tag="ob")
            id_bf = cp.tile([128, 128], BF16, tag="idb")
            id_f32 = cp.tile([128, 128], F32, tag="idf")

            for j in range(2):
                nc.sync.dma_start(w1_t[:, j, :], w1_d[ts(j, 128), :])
                nc.sync.dma_start(w2_t[:, j, :], w2_d[ts(j, 128), :])
                nc.sync.dma_start(w3_t[:, j, :], w3_d[ts(j, 128), :])
                nc.sync.dma_start(b3_t[:, j, :], b3_d[ts(j, 128), :])
            for j in range(4):
                nc.sync.dma_start(ow_t[:, j, :], ow_d[ts(j, 128), :])
                nc.sync.dma_start(pwo_t[:, j, :], pwo_d[ts(j, 128), :])
            nc.sync.dma_start(pw_t[:], pw_d[:, :])
            nc.sync.dma_start(b1_t[:], b1_d[:, :])
            nc.sync.dma_start(b2_t[:], b2_d[:, :])
            nc.sync.dma_start(ob_t[:], ob_d[:, :])
            make_identity(nc, id_bf[:])
            make_identity(nc, id_f32[:])

            # ---- persistent activations ----
            y1_t = pp.tile([128, N], BF16, tag="y1")
            y2_t = pp.tile([128, N], BF16, tag="y2")
            y1tm_t = pp.tile([128, N], BF16, tag="y1tm")
            e_t = pp.tile([128, NPAIRS, N], BF16, tag="elm")
            ycb_t = pp.tile([128, N], BF16, tag="ycb")
            zpart_t = pp.tile([128, NPAIRS, NC_], F32, tag="zpart")
            rinv_t = pp.tile([128, NPAIRS], F32, tag="rinv")

            # ---- phase A (pipelined): cv1 + token-major transpose of y1 ----
            def cv1_chunk(ci):
                c0, w = CHUNKS[ci]
                x_c = xp.tile([128, 2, 512], BF16, tag="x")
                for j in range(2):
                    nc.sync.dma_start(x_c[:, j, :w], x_d[ts(j, 128), c0 : c0 + w])
                p1 = pm.tile([128, 512], F32, tag="pm")
                nc.tensor.matmul(p1[:, :w], w1_t[:, 0, :], x_c[:, 0, :w], start=True, stop=False)
                nc.tensor.matmul(p1[:, :w], w1_t[:, 1, :], x_c[:, 1, :w], start=False, stop=True)
                nc.scalar.activation(y1_t[:, c0 : c0 + w], p1[:, :w], AF.Silu, bias=b1_t[:])
                return x_c

            def y1tm_chunk(ci):
                tp = ptp.tile([128, 512], BF16, tag="tp")
                sub = list(subchunks(ci))
                for k, t in enumerate(sub):
                    nc.tensor.transpose(tp[:, ts(k, 128)], y1_t[:, ts(t, 128)], id_bf[:])
                nc.vector.tensor_copy(
                    y1tm_t[:, sub[0] * 128 : (sub[-1] + 1) * 128], tp[:, : len(sub) * 128]
                )

            def cv2_chunk(ci, x_c):
                c0, w = CHUNKS[ci]
                p2 = pm.tile([128, 512], F32, tag="pm")
                nc.tensor.matmul(p2[:, :w], w2_t[:, 0, :], x_c[:, 0, :w], start=True, stop=False)
                nc.tensor.matmul(p2[:, :w], w2_t[:, 1, :], x_c[:, 1, :w], start=False, stop=True)
                nc.scalar.activation(y2_t[:, c0 : c0 + w], p2[:, :w], AF.Silu, bias=b2_t[:])

            for ci in range(NC_):
                x_c = cv1_chunk(ci)
                cv2_chunk(ci, x_c)
                if ci > 0:
                    y1tm_chunk(ci - 1)
            y1tm_chunk(NC_ - 1)

            # ---- pooling -> rep -> rep_cm -> Q ----
            pool1 = pp.tile([128, 640], F32, tag="pool1")
            nc.vector.tensor_reduce(
                pool1[:],
                y1_t[:].rearrange("p (rw kw c) -> p rw kw c", rw=80, kw=8, c=10),
                axis=mybir.AxisListType.X,
                op=ALU.add,
            )
            pool2 = pp.tile([128, 64], F32, tag="pool2")
            nc.vector.tensor_reduce(
                pool2[:],
                pool1[:].rearrange("p (kh r kw) -> p kh kw r", kh=8, r=10, kw=8),
                axis=mybir.AxisListType.X,
                op=ALU.add,
            )
            y1pool_bf = pp.tile([128, 64], BF16, tag="y1pool")
            nc.vector.tensor_scalar_mul(y1pool_bf[:], pool2[:], 1.0 / 100.0)

            rep_ps = pm.tile([64, 512], F32, tag="pm")
            nc.tensor.matmul(rep_ps[:], y1pool_bf[:], pw_t[:], start=True, stop=True)
            rep_f32 = pp.tile([64, 512], F32, tag="repf")
            rep_bf = pp.tile([64, 512], BF16, tag="repb")
            nc.vector.tensor_copy(rep_f32[:], rep_ps[:])
            nc.vector.tensor_copy(rep_bf[:], rep_ps[:])

            repcm_bd, repcm_f, q_bf = [], [], []
            for pr in range(NPAIRS):
                tpb = ps.tile([128, 128], BF16, tag="lm")
                nc.tensor.transpose(tpb[:, :64], rep_bf[:, ts(pr, 128)], id_bf[:64, :64])
                bd = pp.tile([128, 128], BF16, tag=f"repbd{pr}")
                nc.gpsimd.memset(bd[:], 0.0)
                nc.vector.tensor_copy(bd[0:64, 0:64], tpb[0:64, :64])
                nc.vector.tensor_copy(bd[64:128, 64:128], tpb[64:128, :64])
                repcm_bd.append(bd)

                tpf = ps.tile([128, 128], F32, tag="lm")
                nc.tensor.transpose(tpf[:, :64], rep_f32[:, ts(pr, 128)], id_f32[:64, :64])
                rcf = pp.tile([128, 64], F32, tag=f"repcf{pr}")
                nc.vector.tensor_copy(rcf[:], tpf[:, :64])
                repcm_f.append(rcf)

            for pr in range(NPAIRS):
                qp = ps.tile([128, 128], F32, tag="lm")
                nc.tensor.matmul(qp[:], pwo_t[:, pr, :], repcm_bd[pr][:], start=True, stop=True)
                q = pp.tile([128, 128], BF16, tag=f"q{pr}")
                nc.vector.tensor_copy(q[:], qp[:])
                q_bf.append(q)

            # ---- phase B (pipelined): logits+exp, cv2, E-transpose + T accum ----
            t_acc = pacc.tile([128, NPAIRS, 128], F32, tag="tacc")

            def logits_chunk(ci):
                c0, w = CHUNKS[ci]
                for pr in range(NPAIRS):
                    pl = pm.tile([128, 512], F32, tag="pm")
                    nc.tensor.matmul(pl[:, :w], q_bf[pr][:], y1_t[:, c0 : c0 + w], start=True, stop=True)
                    nc.scalar.activation(
                        e_t[:, pr, c0 : c0 + w],
                        pl[:, :w],
                        AF.Exp,
                        scale=SCALE,
                        accum_out=zpart_t[:, pr, ci : ci + 1],
                    )

            def etp_chunk(ci):
                for t in subchunks(ci):
                    tpe = ptp.tile([128, 512], BF16, tag="tp")
                    for pr in range(NPAIRS):
                        nc.tensor.transpose(tpe[:, ts(pr, 128)], e_t[:, pr, ts(t, 128)], id_bf[:])
                    etm = ep.tile([128, 512], BF16, tag="etm")
                    nc.vector.tensor_copy(etm[:], tpe[:])
                    for pr in range(NPAIRS):
                        nc.tensor.matmul(
                            t_acc[:, pr, :],
                            etm[:, ts(pr, 128)],
                            y1tm_t[:, ts(t, 128)],
                            start=(t == 0),
                            stop=(t == NT - 1),
                        )

            for ci in range(NC_):
                logits_chunk(ci)
                if ci > 0:
                    etp_chunk(ci - 1)
            etp_chunk(NC_ - 1)

            # ---- softmax denominators ----
            for pr in range(NPAIRS):
                nc.vector.tensor_reduce(
                    rinv_t[:, pr : pr + 1], zpart_t[:, pr, :], axis=mybir.AxisListType.X, op=ALU.add
                )
            nc.vector.reciprocal(rinv_t[:], rinv_t[:])

            # ---- landmark-sized attention core (4 pairs interleaved stepwise) ----
            def lm_sb(tag, dtype=BF16):
                return [pp.tile([128, 128], dtype, tag=f"{tag}{pr}", name=f"{tag}{pr}") for pr in range(NPAIRS)]

            tn, tnt, rep2, rep2b = lm_sb("tn"), lm_sb("tnt"), lm_sb("rep2", F32), lm_sb("rep2b")
            e2, e2n, e2t = lm_sb("e2", F32), lm_sb("e2n"), lm_sb("e2t")
            r2l, xd, g_bf = lm_sb("r2l"), lm_sb("xd"), lm_sb("g")
            z2 = [pp.tile([128, 1], F32, tag=f"z2{pr}", name=f"z2{pr}") for pr in range(NPAIRS)]

            for pr in range(NPAIRS):  # rinv * T  (k-double rows)
                nc.vector.tensor_scalar(
                    tn[pr][:], t_acc[:, pr, :], rinv_t[:, pr : pr + 1], None, op0=ALU.mult
                )
            tnt_ps = [ps.tile([128, 128], BF16, tag="lm", name=f"tnt_ps{pr}") for pr in range(NPAIRS)]
            for pr in range(NPAIRS):
                nc.tensor.transpose(tnt_ps[pr][:], tn[pr][:], id_bf[:])
            for pr in range(NPAIRS):
                nc.vector.tensor_copy(tnt[pr][:], tnt_ps[pr][:])
            rd_ps = [ps.tile([128, 128], F32, tag="lm", name=f"rd_ps{pr}") for pr in range(NPAIRS)]
            for pr in range(NPAIRS):  # rep_delta channel-major
                nc.tensor.matmul(rd_ps[pr][:], pw_t[:, ts(pr, 128)], tnt[pr][:], start=True, stop=True)
            for pr in range(NPAIRS):
                nc.gpsimd.memset(rep2[pr][:], 0.0)
            for pr in range(NPAIRS):
                for q in range(2):
                    qs = slice(64 * q, 64 * (q + 1))
                    nc.vector.scalar_tensor_tensor(
                        rep2[pr][qs, qs], rd_ps[pr][qs, qs], sr[2 * pr + q],
                        repcm_f[pr][qs, :], op0=ALU.mult, op1=ALU.add,
                    )
                nc.vector.tensor_copy(rep2b[pr][:], rep2[pr][:])
            l2_ps = [ps.tile([128, 128], F32, tag="lm", name=f"l2_ps{pr}") for pr in range(NPAIRS)]
            for pr in range(NPAIRS):
                nc.tensor.matmul(l2_ps[pr][:], rep2b[pr][:], rep2b[pr][:], start=True, stop=True)
            for pr in range(NPAIRS):
                nc.scalar.activation(e2[pr][:], l2_ps[pr][:], AF.Exp, scale=SCALE)
            for pr in range(NPAIRS):
                nc.vector.tensor_reduce(z2[pr][0:64, :], e2[pr][0:64, 0:64], axis=mybir.AxisListType.X, op=ALU.add)
                nc.vector.tensor_reduce(z2[pr][64:128, :], e2[pr][64:128, 64:128], axis=mybir.AxisListType.X, op=ALU.add)
                nc.vector.reciprocal(z2[pr][:], z2[pr][:])
            for pr in range(NPAIRS):
                for q in range(2):
                    qs = slice(64 * q, 64 * (q + 1))
                    nc.vector.tensor_scalar(
                        e2n[pr][qs, :], e2[pr][qs, :], z2[pr][qs, :], sx[2 * pr + q],
                        op0=ALU.mult, op1=ALU.mult,
                    )
            e2t_ps = [ps.tile([128, 128], BF16, tag="lm", name=f"e2t_ps{pr}") for pr in range(NPAIRS)]
            r2l_ps = [ps.tile([128, 128], BF16, tag="lm", name=f"r2l_ps{pr}") for pr in range(NPAIRS)]
            for pr in range(NPAIRS):
                nc.tensor.transpose(e2t_ps[pr][:], e2n[pr][:], id_bf[:])
                nc.tensor.transpose(r2l_ps[pr][:], rep2b[pr][:], id_bf[:])
            for pr in range(NPAIRS):
                nc.gpsimd.memset(e2t[pr][:], 0.0)
                for q in range(2):
                    qs = slice(64 * q, 64 * (q + 1))
                    nc.vector.tensor_copy(e2t[pr][qs, qs], e2t_ps[pr][qs, qs])
                nc.vector.tensor_copy(r2l[pr][:], r2l_ps[pr][:])
            xd_ps = [ps.tile([128, 128], F32, tag="lm", name=f"xd_ps{pr}") for pr in range(NPAIRS)]
            for pr in range(NPAIRS):  # x_delta channel-major (block-diag)
                nc.tensor.matmul(xd_ps[pr][:], r2l[pr][:], e2t[pr][:], start=True, stop=True)
            for pr in range(NPAIRS):
                nc.vector.tensor_copy(xd[pr][:], xd_ps[pr][:])
            g_ps = [ps.tile([128, 128], F32, tag="lm", name=f"g_ps{pr}") for pr in range(NPAIRS)]
            for pr in range(NPAIRS):
                nc.tensor.matmul(g_ps[pr][:], xd[pr][:], ow_t[:, pr, :], start=True, stop=True)
            for pr in range(NPAIRS):
                nc.vector.tensor_scalar(
                    g_bf[pr][:], g_ps[pr][:], rinv_t[:, pr : pr + 1], None, op0=ALU.mult
                )

            # ---- phase C (pipelined): scatter + bias, then cv3 + SiLU + out ----
            def scatter_chunk(ci):
                c0, w = CHUNKS[ci]
                sc = pm.tile([128, 512], F32, tag="pm")
                for pr in range(NPAIRS):
                    nc.tensor.matmul(
                        sc[:, :w], g_bf[pr][:], e_t[:, pr, c0 : c0 + w],
                        start=(pr == 0), stop=(pr == NPAIRS - 1),
                    )
                nc.vector.tensor_scalar(
                    ycb_t[:, c0 : c0 + w], sc[:, :w], ob_t[:], None, op0=ALU.add
                )

            def cv3_chunk(ci):
                c0, w = CHUNKS[ci]
                for co in range(2):
                    po = pm.tile([128, 512], F32, tag="pm")
                    nc.tensor.matmul(po[:, :w], w3_t[:, 0, ts(co, 128)], ycb_t[:, c0 : c0 + w], start=True, stop=False)
                    nc.tensor.matmul(po[:, :w], w3_t[:, 1, ts(co, 128)], y2_t[:, c0 : c0 + w], start=False, stop=True)
                    ot = op_.tile([128, 512], F32, tag="ot")
                    nc.scalar.activation(ot[:, :w], po[:, :w], AF.Silu, bias=b3_t[:, co, :])
                    nc.sync.dma_start(out_d[ts(co, 128), c0 : c0 + w], ot[:, :w])

            for ci in range(NC_):
                scatter_chunk(ci)
                if ci > 0:
                    cv3_chunk(ci - 1)
            cv3_chunk(NC_ - 1)

    nc.finalize()
    return nc


_CACHE: dict = {}


def _get_nc(step_rep, step_x):
    key = (tuple(np.asarray(step_rep).reshape(-1).tolist()),
           tuple(np.asarray(step_x).reshape(-1).tolist()))
    if key not in _CACHE:
        _CACHE[key] = _build(step_rep, step_x)
    return _CACHE[key]


def run(inputs: dict, trace: bool = False, tmpdir: str | None = None):
    bf = ml_dtypes.bfloat16
    x = np.asarray(inputs["x"], np.float32).reshape(B, C1, N)

    def prep(a):
        return np.ascontiguousarray(np.asarray(a, np.float32)).astype(bf)

    w1t = prep((np.asarray(inputs["cv1_s"], np.float32)[:, None] * np.asarray(inputs["cv1_w"], np.float32)).T)
    w2t = prep((np.asarray(inputs["cv2_s"], np.float32)[:, None] * np.asarray(inputs["cv2_w"], np.float32)).T)
    w3t = prep((np.asarray(inputs["cv3_s"], np.float32)[:, None] * np.asarray(inputs["cv3_w"], np.float32)).T)
    pwt = prep(np.asarray(inputs["proj_w"], np.float32).T)
    pwo = prep(np.asarray(inputs["proj_w"], np.float32))
    owt = prep(np.asarray(inputs["out_w"], np.float32).T)
    b1 = np.ascontiguousarray(np.asarray(inputs["cv1_b"], np.float32).reshape(C_, 1))
    b2 = np.ascontiguousarray(np.asarray(inputs["cv2_b"], np.float32).reshape(C_, 1))
    b3 = np.ascontiguousarray(np.asarray(inputs["cv3_b"], np.float32).reshape(C2, 1))
    ob = np.ascontiguousarray(np.asarray(inputs["out_b"], np.float32).reshape(C_, 1))

    nc = _get_nc(inputs["step_rep"], inputs["step_x"])

    in_maps = []
    for b in range(B):
        in_maps.append(
            {
                "x": np.ascontiguousarray(x[b].astype(bf)),
                "w1t": w1t, "b1": b1,
                "w2t": w2t, "b2": b2,
                "w3t": w3t, "b3": b3,
                "pwt": pwt, "pwo": pwo, "owt": owt, "outb": ob,
            }
        )

    res = run_bass_kernel_spmd(
        nc, in_maps, core_ids=list(range(B)), trace=trace, tmpdir=tmpdir
    )
    out = np.stack([np.asarray(res.results[b]["out"], np.float32) for b in range(B)])
    return out.reshape(B, C2, H, W), res


def kernel(**inputs) -> np.ndarray:
    out, _ = run(inputs, trace=False)
    return out


# revision 23
# speedup vs baseline: 1.4249x; 1.2123x over previous
"""Trainium2 Bass kernel for nn_C3k_CBSA (landmark/CBSA sparse attention block).

Strategy: data-parallel over batch B=8 across 8 NeuronCores (one batch element
per core, zero collectives). Per core the whole block is fused into one Bass
kernel: cv1/cv2 1x1 convs + SiLU, landmark pooling, landmark<->token cross
attention, landmark self attention, scatter-back, output projection, cv3.

Key algebraic restructurings (all exact up to fp assoc.):
  - logits = rep_h.T @ proj_h = (proj_w @ rep_cm).T @ y1  -> proj never
    materialized over tokens; only a tiny per-pair Q = pw.T @ rep_cm.
  - rep = pool(proj) = proj_w @ pool(y1): pooling commutes with 1x1 conv.
  - rep_delta = (E @ y1.T) @ proj_w.T with E transposed chunkwise on PE.
  - softmax 1/Z and step_x folded into landmark-sized tensors (E stays
    unnormalized); scatter-back is G'.T @ E with stacked-landmark contraction.

Head pairing packs two 64-dim heads into 128 partitions with block-diagonal
stationary operands so every matmul uses the full PE array. Emission is
software-pipelined (lag-one chunk) so each engine's in-order queue never
stalls on the previous chunk's cross-engine dependency.
"""

import os
import numpy as np
import ml_dtypes

try:
    import concourse  # noqa: F401
except ImportError:  # fresh grading dir: fall back to the staged repo path
    import sys

    for p in ("/opt/trn_rl_repo", "/root/.axon_site/_ro/trn_rl_repo"):
        if os.path.isdir(p):
            sys.path.insert(0, p)
            break

import concourse.bass as bass
import concourse.mybir as mybir
import concourse.tile as tile
from concourse import bacc
from concourse.bass import ts
from concourse.bass_utils import run_bass_kernel_spmd
from concourse.masks import make_identity

F32 = mybir.dt.float32
BF16 = mybir.dt.bfloat16
AF = mybir.ActivationFunctionType
ALU = mybir.AluOpType

B, C1, C2, H, W = 8, 256, 256, 80, 80
C_ = 128
HEADS, DH = 8, 64
INNER = HEADS * DH  # 512
SCALE = DH ** -0.5
N = H * W  # 6400
NPAIRS = HEADS // 2  # 4 head-pair groups of 128 partitions

CHUNKS = [(i * 1024, min(1024, N - i * 1024)) for i in range((N + 1023) // 1024)]
NC_ = len(CHUNKS)  # 7 (6x1024 + 256)


def halves(w):
    return [(o, min(512, w - o)) for o in range(0, w, 512)]
NT = N // 128  # 50 token chunks of 128


def _build(step_rep: np.ndarray, step_x: np.ndarray) -> bass.Bass:
    nc = bacc.Bacc("TRN2", target_bir_lowering=False, debug=False, num_devices=8)

    x_d = nc.dram_tensor("x", [C1, N], BF16, kind="ExternalInput")
    wb_d = nc.dram_tensor("wb", [128, 2560], BF16, kind="ExternalInput")
    wf_d = nc.dram_tensor("wf", [128, 524], F32, kind="ExternalInput")
    out_d = nc.dram_tensor("out", [C2, N], F32, kind="ExternalOutput")

    sr = [float(v) for v in np.asarray(step_rep).reshape(-1)]
    sx = [float(v) for v in np.asarray(step_x).reshape(-1)]

    def subchunks(ci):
        c0, w = CHUNKS[ci]
        return range(c0 // 128, (c0 + w) // 128)

    with tile.TileContext(nc) as tc:
        with (
            tc.tile_pool(name="const", bufs=1) as cp,
            tc.tile_pool(name="persist", bufs=1) as pp,
            tc.tile_pool(name="etm", bufs=16) as ep,
            tc.tile_pool(name="outs", bufs=4) as op_,
            tc.tile_pool(name="pmain", bufs=3, space="PSUM") as pm,
            tc.tile_pool(name="pscat", bufs=1, space="PSUM") as psc,
            tc.tile_pool(name="psmall", bufs=1, space="PSUM") as ps,
        ):
            # ---- constants: one bf16 blob + one f32 blob, x persistent ----
            wb_t = cp.tile([128, 2560], BF16, tag="wb")
            wf_t = cp.tile([128, 524], F32, tag="wf")
            id_bf = cp.tile([128, 128], BF16, tag="idb")
            id_f32 = cp.tile([128, 128], F32, tag="idf")
            x_t = cp.tile([128, 2, N], BF16, tag="xt")

            # PE warm-up during the input-DMA window: memset a dummy weight
            # tile first on gpsimd (before its DMA triggers), then spam
            # matmuls so the HAM clock-gate opens before real work arrives
            wid = cp.tile([128, 128], BF16, tag="wid")
            nc.gpsimd.memset(wid[:], 1.0)
            for wi in range(64):
                wp = pm.tile([128, 128], F32, tag="pm", name=f"warm{wi}")
                nc.tensor.matmul(wp[:], wid[:], wid[:], start=True, stop=True)

            nc.sync.dma_start(wb_t[:], wb_d[:, :])
            QN = N // 4
            for h in range(4):
                sl = slice(h * QN, (h + 1) * QN)
                nc.sync.dma_start(x_t[:, 0, sl], x_d[0:128, sl])
                nc.gpsimd.dma_start(x_t[:, 1, sl], x_d[128:256, sl])
                if h == 0:
                    nc.gpsimd.dma_start(wf_t[:], wf_d[:, :])
            make_identity(nc, id_bf[:])
            make_identity(nc, id_f32[:])

            def W1(j):
                return wb_t[:, j * 128 : (j + 1) * 128]

            def W2(j):
                return wb_t[:, 256 + j * 128 : 256 + (j + 1) * 128]

            def W3(j, co):
                o = 512 + j * 256 + co * 128
                return wb_t[:, o : o + 128]

            PWfull = wb_t[:, 1024:1536]

            def PW(pr):
                return wb_t[:, 1024 + pr * 128 : 1024 + (pr + 1) * 128]

            def PWO(pr):
                return wb_t[:, 1536 + pr * 128 : 1536 + (pr + 1) * 128]

            def OW(pr):
                return wb_t[:, 2048 + pr * 128 : 2048 + (pr + 1) * 128]

            b1_a = wf_t[:, 0:1]
            b2_a = wf_t[:, 1:2]
            ob_a = wf_t[:, 4:5]

            def B3(co):
                return wf_t[:, 2 + co : 3 + co]

            srm = wf_t[:, 8:520].rearrange("p (a b) -> p a b", a=4)
            sxv = wf_t[:, 520:524]

            # ---- persistent activations ----
            y1_t = pp.tile([128, N], BF16, tag="y1")
            y2_t = pp.tile([128, N], BF16, tag="y2")
            y1tm_t = pp.tile([128, N], BF16, tag="y1tm")
            e_t = pp.tile([128, NPAIRS, N], BF16, tag="elm")
            ycb_t = pp.tile([128, N], BF16, tag="ycb")
            zpart_t = pp.tile([128, NPAIRS, NC_], F32, tag="zpart")
            rinv_t = pp.tile([128, NPAIRS], F32, tag="rinv")

            # ---- phase A (pipelined): cv1 + token-major transpose of y1 ----
            def cv1_chunk(ci):
                c0, w = CHUNKS[ci]
                p1 = pm.tile([128, 1024], F32, tag="pm")
                for o, hw in halves(w):
                    nc.tensor.matmul(p1[:, o : o + hw], W1(0), x_t[:, 0, c0 + o : c0 + o + hw], start=True, stop=False)
                    nc.tensor.matmul(p1[:, o : o + hw], W1(1), x_t[:, 1, c0 + o : c0 + o + hw], start=False, stop=True)
                nc.scalar.activation(y1_t[:, c0 : c0 + w], p1[:, :w], AF.Silu, bias=b1_a)

            y1tm_3d = y1tm_t[:].rearrange("p (t c) -> p t c", c=128)

            def y1tm_chunk(ci):
                c0, w = CHUNKS[ci]
                sub = list(subchunks(ci))
                nc.sync.dma_start_transpose(
                    y1tm_3d[:, sub[0] : sub[-1] + 1, :], y1_t[:, c0 : c0 + w]
                )

            def cv2_chunk(ci):
                c0, w = CHUNKS[ci]
                p2 = pm.tile([128, 1024], F32, tag="pm")
                for o, hw in halves(w):
                    nc.tensor.matmul(p2[:, o : o + hw], W2(0), x_t[:, 0, c0 + o : c0 + o + hw], start=True, stop=False)
                    nc.tensor.matmul(p2[:, o : o + hw], W2(1), x_t[:, 1, c0 + o : c0 + o + hw], start=False, stop=True)
                nc.scalar.activation(y2_t[:, c0 : c0 + w], p2[:, :w], AF.Silu, bias=b2_a)

            # pooling pass 1, split into 5 row-groups emitted as soon as the
            # covering cv1 chunks are done (keeps it off the critical path)
            pool1 = pp.tile([128, 640], F32, tag="pool1")

            def pool1_piece(r):
                nc.vector.tensor_reduce(
                    pool1[:, r * 128 : (r + 1) * 128],
                    y1_t[:, r * 1280 : (r + 1) * 1280].rearrange(
                        "p (rw kw c) -> p rw kw c", rw=16, kw=8, c=10
                    ),
                    axis=mybir.AxisListType.X,
                    op=ALU.add,
                )

            piece_after = {1: 0, 2: 1, 3: 2, 4: 3, 6: 4}
            for ci in range(NC_):
                cv1_chunk(ci)
                if ci > 0:
                    y1tm_chunk(ci - 1)
                if ci in piece_after:
                    pool1_piece(piece_after[ci])
            y1tm_chunk(NC_ - 1)

            # ---- pooling pass 2 -> rep -> rep_cm -> Q ----
            pool2 = pp.tile([128, 64], F32, tag="pool2")
            nc.vector.tensor_reduce(
                pool2[:],
                pool1[:].rearrange("p (kh r kw) -> p kh kw r", kh=8, r=10, kw=8),
                axis=mybir.AxisListType.X,
                op=ALU.add,
            )
            y1pool_bf = pp.tile([128, 64], BF16, tag="y1pool")
            nc.vector.tensor_scalar_mul(y1pool_bf[:], pool2[:], 1.0 / 100.0)

            for ci in range(NC_):
                cv2_chunk(ci)

            rep_ps = pm.tile([64, 512], F32, tag="pm")
            nc.tensor.matmul(rep_ps[:], y1pool_bf[:], PWfull, start=True, stop=True)
            rep_f32 = pp.tile([64, 512], F32, tag="repf")
            rep_bf = pp.tile([64, 512], BF16, tag="repb")
            nc.vector.tensor_copy(rep_f32[:], rep_ps[:])
            nc.vector.tensor_copy(rep_bf[:], rep_ps[:])

            tpb_m = ps.tile([128, 4, 64], BF16, tag="lm", name="tpb_m")
            for pr in range(NPAIRS):
                nc.tensor.transpose(tpb_m[:, pr, :], rep_bf[:, ts(pr, 128)], id_bf[:64, :64])
            bd_m = pp.tile([128, 4, 128], BF16, tag="bd_m")
            nc.gpsimd.memset(bd_m[:], 0.0)
            nc.vector.tensor_copy(bd_m[0:64, :, 0:64], tpb_m[0:64, :, :])
            nc.vector.tensor_copy(bd_m[64:128, :, 64:128], tpb_m[64:128, :, :])

            tpf_m = ps.tile([128, 4, 64], F32, tag="lm", name="tpf_m")
            for pr in range(NPAIRS):
                nc.tensor.transpose(tpf_m[:, pr, :], rep_f32[:, ts(pr, 128)], id_f32[:64, :64])
            repcm_m = pp.tile([128, 4, 128], F32, tag="repcm_m")
            nc.gpsimd.memset(repcm_m[:], 0.0)
            nc.vector.tensor_copy(repcm_m[0:64, :, 0:64], tpf_m[0:64, :, :])
            nc.vector.tensor_copy(repcm_m[64:128, :, 64:128], tpf_m[64:128, :, :])

            qp_m = ps.tile([128, 4, 128], F32, tag="lm", name="qp_m")
            for pr in range(NPAIRS):
                nc.tensor.matmul(qp_m[:, pr, :], PWO(pr), bd_m[:, pr, :], start=True, stop=True)
            q_m = pp.tile([128, 4, 128], BF16, tag="q_m")
            nc.vector.tensor_copy(q_m[:], qp_m[:])

            # ---- phase B (pipelined): logits+exp, cv2, E-transpose + T accum ----
            t_acc = ps.tile([128, NPAIRS, 128], F32, tag="lm", name="t_acc")

            etms = {}

            def logits_pair(ci, pr):
                c0, w = CHUNKS[ci]
                pl = pm.tile([128, 1024], F32, tag="pm")
                for o, hw in halves(w):
                    nc.tensor.matmul(pl[:, o : o + hw], q_m[:, pr, :], y1_t[:, c0 + o : c0 + o + hw], start=True, stop=True)
                nc.scalar.activation(
                    e_t[:, pr, c0 : c0 + w], pl[:, :w], AF.Exp, scale=SCALE
                )
                nc.vector.tensor_reduce(
                    zpart_t[:, pr, ci : ci + 1],
                    e_t[:, pr, c0 : c0 + w],
                    axis=mybir.AxisListType.X,
                    op=ALU.add,
                )
                etm = ep.tile([128, 8, 128], BF16, tag="etm")
                nc.sync.dma_start_transpose(
                    etm[:, : w // 128, :], e_t[:, pr, c0 : c0 + w]
                )
                etms[(ci, pr)] = etm

            def tmm_group(ci, pr):
                etm = etms.pop((ci, pr))
                for k, t in enumerate(subchunks(ci)):
                    nc.tensor.matmul(
                        t_acc[:, pr, :],
                        etm[:, k, :],
                        y1tm_t[:, ts(t, 128)],
                        start=(t == 0),
                        stop=(t == NT - 1),
                    )

            for ci in range(NC_):
                for pr in range(NPAIRS):
                    logits_pair(ci, pr)
                    if ci > 1:
                        tmm_group(ci - 2, pr)
            for ci in (NC_ - 2, NC_ - 1):
                for pr in range(NPAIRS):
                    tmm_group(ci, pr)

            # ---- softmax denominators ----
            for pr in range(NPAIRS):
                nc.vector.tensor_reduce(
                    rinv_t[:, pr : pr + 1], zpart_t[:, pr, :], axis=mybir.AxisListType.X, op=ALU.add
                )
            nc.vector.reciprocal(rinv_t[:], rinv_t[:])

            # ---- landmark-sized attention core (pairs batched in master tiles) ----
            tn_m = pp.tile([128, 4, 128], BF16, tag="tn_m")
            nc.vector.tensor_tensor(
                tn_m[:], t_acc[:], rinv_t[:, :, None].to_broadcast((128, 4, 128)), op=ALU.mult
            )
            tnt_ps = ps.tile([128, 4, 128], BF16, tag="lm", name="tnt_ps")
            for pr in range(NPAIRS):
                nc.tensor.transpose(tnt_ps[:, pr, :], tn_m[:, pr, :], id_bf[:])
            tnt_m = pp.tile([128, 4, 128], BF16, tag="tnt_m")
            nc.vector.tensor_copy(tnt_m[:], tnt_ps[:])

            rd_ps = ps.tile([128, 4, 128], F32, tag="lm", name="rd_ps")
            for pr in range(NPAIRS):  # rep_delta channel-major
                nc.tensor.matmul(rd_ps[:, pr, :], PW(pr), tnt_m[:, pr, :], start=True, stop=True)

            rep2_m = pp.tile([128, 4, 128], F32, tag="rep2_m")
            nc.vector.tensor_tensor(rep2_m[:], rd_ps[:], srm, op=ALU.mult)
            nc.vector.tensor_add(rep2_m[:], rep2_m[:], repcm_m[:])
            rep2b_m = pp.tile([128, 4, 128], BF16, tag="rep2b_m")
            nc.vector.tensor_copy(rep2b_m[:], rep2_m[:])

            l2_ps = ps.tile([128, 4, 128], F32, tag="lm", name="l2_ps")
            for pr in range(NPAIRS):
                nc.tensor.matmul(l2_ps[:, pr, :], rep2b_m[:, pr, :], rep2b_m[:, pr, :], start=True, stop=True)
            e2_m = pp.tile([128, 4, 128], F32, tag="e2_m")
            nc.scalar.activation(e2_m[:], l2_ps[:], AF.Exp, scale=SCALE)

            z2_m = pp.tile([128, 4], F32, tag="z2_m")
            nc.vector.tensor_reduce(z2_m[0:64, :], e2_m[0:64, :, 0:64], axis=mybir.AxisListType.X, op=ALU.add)
            nc.vector.tensor_reduce(z2_m[64:128, :], e2_m[64:128, :, 64:128], axis=mybir.AxisListType.X, op=ALU.add)
            nc.vector.reciprocal(z2_m[:], z2_m[:])

            zsx_m = pp.tile([128, 4], F32, tag="zsx_m")
            nc.vector.tensor_mul(zsx_m[:], z2_m[:], sxv)
            e2n_m = pp.tile([128, 4, 128], BF16, tag="e2n_m")
            nc.vector.tensor_tensor(
                e2n_m[:], e2_m[:], zsx_m[:, :, None].to_broadcast((128, 4, 128)), op=ALU.mult
            )
            tr_ps = ps.tile([128, 8, 128], BF16, tag="lm", name="tr_ps")
            e2t_ps = tr_ps[:, 0:4, :]
            r2l_ps = tr_ps[:, 4:8, :]
            for pr in range(NPAIRS):
                nc.tensor.transpose(e2t_ps[:, pr, :], e2n_m[:, pr, :], id_bf[:])
                nc.tensor.transpose(r2l_ps[:, pr, :], rep2b_m[:, pr, :], id_bf[:])
            e2t_m = pp.tile([128, 4, 128], BF16, tag="e2t_m")
            nc.gpsimd.memset(e2t_m[:], 0.0)
            nc.vector.tensor_copy(e2t_m[0:64, :, 0:64], e2t_ps[0:64, :, 0:64])
            nc.vector.tensor_copy(e2t_m[64:128, :, 64:128], e2t_ps[64:128, :, 64:128])
            r2l_m = pp.tile([128, 4, 128], BF16, tag="r2l_m")
            nc.vector.tensor_copy(r2l_m[:], r2l_ps[:])

            xd_ps = ps.tile([128, 4, 128], F32, tag="lm", name="xd_ps")
            for pr in range(NPAIRS):  # x_delta channel-major (block-diag)
                nc.tensor.matmul(xd_ps[:, pr, :], r2l_m[:, pr, :], e2t_m[:, pr, :], start=True, stop=True)
            xd_m = pp.tile([128, 4, 128], BF16, tag="xd_m")
            nc.vector.tensor_copy(xd_m[:], xd_ps[:])

            g_ps = ps.tile([128, 4, 128], F32, tag="lm", name="g_ps")
            for pr in range(NPAIRS):
                nc.tensor.matmul(g_ps[:, pr, :], xd_m[:, pr, :], OW(pr), start=True, stop=True)
            g_m = pp.tile([128, 4, 128], BF16, tag="g_m")
            nc.vector.tensor_tensor(
                g_m[:], g_ps[:], rinv_t[:, :, None].to_broadcast((128, 4, 128)), op=ALU.mult
            )

            # ---- phase C (pipelined): scatter + bias, then cv3 + SiLU + out ----
            def scatter_chunk(ci):
                c0, w = CHUNKS[ci]
                for o, hw in halves(w):
                    sc = psc.tile([128, 512], F32, tag="sc")
                    for pr in range(NPAIRS):
                        nc.tensor.matmul(
                            sc[:, :hw], g_m[:, pr, :], e_t[:, pr, c0 + o : c0 + o + hw],
                            start=(pr == 0), stop=(pr == NPAIRS - 1),
                        )
                    nc.vector.tensor_scalar(
                        ycb_t[:, c0 + o : c0 + o + hw], sc[:, :hw], ob_a, None, op0=ALU.add
                    )

            def cv3_chunk(ci):
                c0, w = CHUNKS[ci]
                for co in range(2):
                    po = pm.tile([128, 1024], F32, tag="pm")
                    for o, hw in halves(w):
                        nc.tensor.matmul(po[:, o : o + hw], W3(0, co), ycb_t[:, c0 + o : c0 + o + hw], start=True, stop=False)
                        nc.tensor.matmul(po[:, o : o + hw], W3(1, co), y2_t[:, c0 + o : c0 + o + hw], start=False, stop=True)
                    ot = op_.tile([128, 1024], F32, tag="ot")
                    nc.scalar.activation(ot[:, :w], po[:, :w], AF.Silu, bias=B3(co))
                    nc.gpsimd.dma_start(out_d[ts(co, 128), c0 : c0 + w], ot[:, :w])

            for ci in range(NC_):
                scatter_chunk(ci)
                if ci > 0:
                    cv3_chunk(ci - 1)
            cv3_chunk(NC_ - 1)

    nc.finalize()
    return nc


_CACHE: dict = {}


def _get_nc(step_rep, step_x):
    key = (tuple(np.asarray(step_rep).reshape(-1).tolist()),
           tuple(np.asarray(step_x).reshape(-1).tolist()))
    if key not in _CACHE:
        _CACHE[key] = _build(step_rep, step_x)
    return _CACHE[key]


def run(inputs: dict, trace: bool = False, tmpdir: str | None = None):
    bf = ml_dtypes.bfloat16
    x = np.asarray(inputs["x"], np.float32).reshape(B, C1, N)

    def pack2(a):  # (K, M) row-major -> (128, K/128*M) with [p, j*M+m] = a[j*128+p, m]
        K, M = a.shape
        return a.reshape(K // 128, 128, M).transpose(1, 0, 2).reshape(128, -1)

    w1t = (np.asarray(inputs["cv1_s"], np.float32)[:, None] * np.asarray(inputs["cv1_w"], np.float32)).T
    w2t = (np.asarray(inputs["cv2_s"], np.float32)[:, None] * np.asarray(inputs["cv2_w"], np.float32)).T
    w3t = (np.asarray(inputs["cv3_s"], np.float32)[:, None] * np.asarray(inputs["cv3_w"], np.float32)).T
    pw = np.asarray(inputs["proj_w"], np.float32)  # (INNER, C_)
    ow = np.asarray(inputs["out_w"], np.float32)  # (C_, INNER)

    wb = np.concatenate(
        [pack2(w1t), pack2(w2t), pack2(w3t), pw.T, pack2(pw), pack2(ow.T)], axis=1
    )
    assert wb.shape == (128, 2560)
    wb = np.ascontiguousarray(wb.astype(bf))

    wf = np.zeros((128, 524), np.float32)
    wf[:, 0] = np.asarray(inputs["cv1_b"], np.float32)
    wf[:, 1] = np.asarray(inputs["cv2_b"], np.float32)
    b3 = np.asarray(inputs["cv3_b"], np.float32)
    wf[:, 2] = b3[0:128]
    wf[:, 3] = b3[128:256]
    wf[:, 4] = np.asarray(inputs["out_b"], np.float32)
    sr = np.asarray(inputs["step_rep"], np.float32).reshape(-1)
    sx = np.asarray(inputs["step_x"], np.float32).reshape(-1)
    p = np.arange(128)
    half = p // 64  # quadrant of each partition
    srmask = np.zeros((128, 4, 128), np.float32)
    for pr in range(4):
        for q in range(2):
            rows = slice(64 * q, 64 * (q + 1))
            cols = slice(64 * q, 64 * (q + 1))
            srmask[rows, pr, cols] = sr[2 * pr + q]
    wf[:, 8:520] = srmask.reshape(128, 512)
    for pr in range(4):
        wf[:, 520 + pr] = sx[2 * pr + half]
    wf = np.ascontiguousarray(wf)

    nc = _get_nc(inputs["step_rep"], inputs["step_x"])

    in_maps = []
    for b in range(B):
        in_maps.append({"x": np.ascontiguousarray(x[b].astype(bf)), "wb": wb, "wf": wf})

    res = run_bass_kernel_spmd(
        nc, in_maps, core_ids=list(range(B)), trace=trace, tmpdir=tmpdir
    )
    out = np.stack([np.asarray(res.results[b]["out"], np.float32) for b in range(B)])
    return out.reshape(B, C2, H, W), res


def kernel(**inputs) -> np.ndarray:
    out, _ = run(inputs, trace=False)
    return out
